# revision 43
# baseline (speedup 1.0000x reference)
"""Trainium2 Bass kernel for nn_Decoder (pre-LN transformer decoder layer).

Sharding: 8 cores = 4 batches x 2 sequence-halves. Core pid -> (batch=pid//2,
s=pid%2). s=0 handles query tokens [0,T0), s=1 handles [T0,L). Each core
computes k/v for its key range on its own (s=1 recomputes the prefix
projections), so no collectives are needed; the host concatenates outputs.

Structure (measured ~460us HW vs ~480-490us for the phase-separated bf16
baseline; rel err ~1.05e-2 vs the 2e-2 gate): ONE continuous instruction
stream. All work besides the attention
span/pair/group skeleton is packaged as closures in two queues:
- ab_q: LN1 + qkv-projection units (x-load, stats, DVE-rsqrt, normalize,
  PE-transpose to hT, q/k matmul+drain, v matmul+drain) per 512-token span.
- ffn_q: LN2 + FFN units per finished span (stats, DVE-rsqrt, normalize,
  transpose, ff1 i-blocks, ff2 token-blocks -> pre-gelu staging in DRAM).
The attention group loop force-pops ab_q until its key-range is covered, and
otherwise pumps a few units after each exp call, so the PE always has dense
matmul work while ACT runs exp: no idle -> HAM clock gate stays at 2.4 GHz,
and the kernel runs at ~max(PE, ACT) rather than the phase sum.

Numerics: PV and w_o are fp8e4 DoubleRow (2 contraction tiles/pass; exp
output, v, attnT, q/k in fp8 -- softmax weights here are ~1.0 +- 0.15 and
numerator/denominator share the quantized values so the error ~cancels).
qkv-projection and FFN matmuls stay bf16 (fp8 there costs ~2e-2 alone). Both
LayerNorm rstds use a DVE cubic+Newton rsqrt so the ACT exp table is never
evicted mid-stream; gelu runs once in a tail through a DRAM staging buffer.

Weight residency is staggered: qkv weights (right stack) are freed once the
last qkv unit is emitted, and only then are the FFN weights fetched.
"""
import collections
import os
import sys

sys.path.insert(0, "/opt/trn_rl_repo")

import contextlib

import ml_dtypes
import numpy as np

import concourse.bass as bass
import concourse.mybir as mybir
import concourse.tile as tile
import concourse.tile_utils as tile_utils
from concourse import bacc
from concourse.bass_utils import run_bass_kernel_spmd
from concourse.masks import make_identity

# trn2 has 224KB/partition physical, ~208 usable; default cap is stale 192.
tile_utils.max_sbuf_usage = 206 * 1024

F32 = mybir.dt.float32
BF16 = mybir.dt.bfloat16
F8 = mybir.dt.float8e4
AF = mybir.ActivationFunctionType
ALU = mybir.AluOpType
DR = mybir.MatmulPerfMode.DoubleRow

if os.environ.get("DECODER_DIMS"):
    B, L, D, H, I, T0 = (int(v) for v in os.environ["DECODER_DIMS"].split(","))
else:
    B, L, D, H, I, T0 = 4, 2048, 768, 12, 3072, 1280
HD = 64
T1 = L - T0
EPS = 1e-5
N_CORES = 2 * B
ND = D // 128
NI = I // 128
NH = H
BN_SUB = 256
WS = 32.0          # host-side fp8 w_o prescale
AS = 16.0          # attnT fp8 prescale

# cubic fit of v^-1/2 on [0.5, 3] (Chebyshev nodes); one Newton step after
# brings rel err to ~1e-4 -- LN rstd on DVE so the ACT exp table is never
# evicted mid-stream.
_nodes = 1.75 + 1.25 * np.cos(np.pi * (np.arange(64) + 0.5) / 64)
_RC = np.polyfit(_nodes, _nodes ** -0.5, 3)  # [c3, c2, c1, c0]


def attn_spans(q_start, q_len, span=512):
    out = []
    q0 = q_start
    while q0 < q_start + q_len:
        w = min(span, q_start + q_len - q0)
        out.append((q0, w, q0 // 128))
        q0 += w
    return out


def build_body(nc, tc, ctx, io, q_start, q_len, kv_len, s_idx):
    (x, wqkb, wvb, wo8, w1b, w2b, bq_t, bk_t, bv, bo, b1t, b2, out) = io
    NT_KV = kv_len // 128
    NT_Q = q_len // 128
    NP = NH // 2
    Dh = D // 2
    c3_, c2_, c1_, c0_ = (float(v) for v in _RC)

    # ---------------- constant tiles ----------------
    consts = ctx.enter_context(tc.tile_pool(name="consts", bufs=1))
    ident = consts.tile([128, 128], BF16, tag="ident")
    make_identity(nc, ident[:])
    # tri[i, j] = 1 where query col j >= key row i (causal keep), else 0.
    tri = consts.tile([128, 128], BF16, tag="tri")
    nc.vector.memset(tri[:], 1.0)
    nc.gpsimd.affine_select(
        out=tri[:], in_=tri[:], pattern=[[1, 128]],
        channel_multiplier=-1, base=0, compare_op=ALU.is_ge, fill=0.0)

    def bcast(vec_ap, n, name, dtype=F32):
        t = consts.tile([128, n], dtype, tag=name)
        src = bass.AP(tensor=vec_ap.tensor, offset=vec_ap.offset,
                      ap=[[0, 128]] + vec_ap.ap)
        nc.gpsimd.dma_start(out=t[:], in_=src)
        return t

    bo_bc = bcast(bo, D, "bo_bc")          # f32: residual path
    b2_bc = bcast(b2, D, "b2_bc")          # f32: pre-gelu
    bv_bc = bcast(bv, NH * HD, "bv_bc")    # f32: v bias (head-major)

    # long-lived activations (left stack)
    oa_pool = ctx.enter_context(tc.tile_pool(name="oa_pool", bufs=1))
    h2_pool = ctx.enter_context(tc.tile_pool(name="h2_pool", bufs=1))
    at_pool = ctx.enter_context(tc.tile_pool(name="attn_pool", bufs=1))
    qkv_pool = ctx.enter_context(tc.tile_pool(name="qkv_pool", bufs=1))
    # small weights needed before the qkv->FFN weight switch
    wsm_pool = ctx.enter_context(tc.tile_pool(name="wsmall", bufs=1))
    wo_sb = wsm_pool.tile([128, ND, D], F8, tag="wo_sb")
    nc.sync.dma_start(out=wo_sb[:], in_=wo8[:, :])
    b1_sb = wsm_pool.tile([128, NI], F32, tag="b1_sb")
    nc.sync.dma_start(out=b1_sb[:], in_=b1t[:, :])

    # w1 prefetched from the start (bottom of the right stack so it stays
    # when the qkv weights above it are freed mid-stream).
    w1p_pool = ctx.enter_context(tc.tile_pool(name="w1p", bufs=1,
                                              side="right"))
    w1_sb = w1p_pool.tile([128, ND, I], BF16, tag="w1_sb")
    nc.scalar.dma_start(out=w1_sb[:], in_=w1b[:, :])
    # qkv weights (fp8, host-prescaled x32) -- freed mid-stream, replaced by
    # the remaining FFN weights once the last qkv unit has been emitted.
    wearly_cm = tc.tile_pool(name="wearly", bufs=1, side="right")
    wearly = wearly_cm.__enter__()
    wqk = wearly.tile([128, ND, NP * 256], F8, tag="wqk")
    nc.sync.dma_start(out=wqk[:], in_=wqkb[:, :])
    wv = wearly.tile([128, ND, NH * HD], F8, tag="wv")
    nc.sync.dma_start(out=wv[:], in_=wvb[:, :])
    bqp = wearly.tile([128, NP], F32, tag="bqp")
    nc.sync.dma_start(out=bqp[:], in_=bq_t[:, :])
    bkp = wearly.tile([128, NP], F32, tag="bkp")
    nc.sync.dma_start(out=bkp[:], in_=bk_t[:, :])
    htp_cm = tc.tile_pool(name="htp", bufs=2, side="right")
    htp = htp_cm.__enter__()

    # head h at partition half 64*(h%2), pair h//2, in both qT2 and kT2.
    qT2 = qkv_pool.tile([128, NP, q_len], F8, tag="qT2")
    kT2 = qkv_pool.tile([128, NP, kv_len], F8, tag="kT2")
    # v token-major: per (tok-window, head) a [128, 128] block; even heads
    # [v | ones], odd heads [ones | v] so the PV output of head 2j+1 lands
    # values on PSUM partitions 64:128 (keeps softmax-drain DVE ops aligned).
    vaug = qkv_pool.tile([128, NT_KV, NH, 128], F8, tag="vaug")
    attnT = at_pool.tile([128, ND, q_len], F8, tag="attnT")
    oaT = oa_pool.tile([128, NT_Q, D], BF16, tag="oaT")

    with contextlib.ExitStack() as phC:
        sps = phC.enter_context(tc.tile_pool(name="sps", bufs=1, space="PSUM"))
        ops_ = phC.enter_context(
            tc.tile_pool(name="ops", bufs=2, space="PSUM"))
        accp = phC.enter_context(
            tc.tile_pool(name="accp", bufs=1, space="PSUM"))
        epool = phC.enter_context(tc.tile_pool(name="epool", bufs=2))
        rpool = phC.enter_context(tc.tile_pool(name="rpool", bufs=2))
        xpool = phC.enter_context(tc.tile_pool(name="xpool", bufs=2))
        lnp = phC.enter_context(tc.tile_pool(name="lnp", bufs=2))
        gbp = phC.enter_context(tc.tile_pool(name="gbp", bufs=2))
        g2pool = phC.enter_context(
            tc.tile_pool(name="g2pool", bufs=1, space="DRAM"))

        mv2 = lnp.tile([128, NT_Q, 2], F32, tag="ln2_mv2", bufs=1)
        rsds = lnp.tile([128, NT_Q], F32, tag="ln2_rsds", bufs=1)
        g2d = g2pool.tile([q_len, D], BF16, tag="g2d")

        def rsqrt_cols(dst, var_ap, ncols):
            """dst[:, 0:ncols] = (var+EPS)^-1/2, DVE-only (cubic + Newton)."""
            vt = lnp.tile([128, 4], F32, tag="rsq_v")
            t = lnp.tile([128, 4], F32, tag="rsq_t")
            a = lnp.tile([128, 4], F32, tag="rsq_a")
            nc.vector.tensor_scalar_add(out=vt[:, 0:ncols], in0=var_ap,
                                        scalar1=EPS)
            nc.vector.tensor_scalar(out=t[:, 0:ncols], in0=vt[:, 0:ncols],
                                    scalar1=c3_, scalar2=c2_,
                                    op0=ALU.mult, op1=ALU.add)
            nc.vector.tensor_tensor(out=t[:, 0:ncols], in0=t[:, 0:ncols],
                                    in1=vt[:, 0:ncols], op=ALU.mult)
            nc.vector.tensor_scalar_add(out=t[:, 0:ncols], in0=t[:, 0:ncols],
                                        scalar1=c1_)
            nc.vector.tensor_tensor(out=t[:, 0:ncols], in0=t[:, 0:ncols],
                                    in1=vt[:, 0:ncols], op=ALU.mult)
            nc.vector.tensor_scalar_add(out=t[:, 0:ncols], in0=t[:, 0:ncols],
                                        scalar1=c0_)
            nc.vector.tensor_tensor(out=a[:, 0:ncols], in0=t[:, 0:ncols],
                                    in1=t[:, 0:ncols], op=ALU.mult)
            nc.vector.scalar_tensor_tensor(
                out=a[:, 0:ncols], in0=vt[:, 0:ncols], scalar=-0.5,
                in1=a[:, 0:ncols], op0=ALU.mult, op1=ALU.mult)
            nc.vector.scalar_tensor_tensor(
                out=dst, in0=a[:, 0:ncols], scalar=1.5,
                in1=t[:, 0:ncols], op0=ALU.add, op1=ALU.mult)

        # ---------------- A/B units: LN1 + qkv per 512-token span ----------
        # scores need q/k coverage; PV (one group later) needs v coverage --
        # separate queues so v-projection units act as exp-gap filler even
        # when a span's scores force all q/k units out up front (s=1).
        qk_q = collections.deque()   # entries: (closure, qk_cov_after)
        v_q = collections.deque()    # entries: (closure, v_cov_after)

        def make_ab_units(sp0, w):
            ntile = w // 128
            st = {}

            def u_ln(twl):
                def run():
                    if 'hT' not in st:
                        st['hT'] = htp.tile([128, ND, 512], F8,
                                            tag="hTs", name="hTs")
                    tw = sp0 // 128 + twl
                    x_t = xpool.tile([128, D], F32, tag="x_t")
                    nc.sync.dma_start(out=x_t[:],
                                      in_=x[tw * 128:(tw + 1) * 128, :])
                    stats = lnp.tile([128, D // BN_SUB, 6], F32,
                                     tag="ln1_stats")
                    xs = x_t[:].rearrange("p (s c) -> p s c", c=BN_SUB)
                    for sgi in range(D // BN_SUB):
                        nc.vector.bn_stats(out=stats[:, sgi, :],
                                           in_=xs[:, sgi, :])
                    mv = lnp.tile([128, 2], F32, tag="ln1_mv")
                    nc.vector.bn_aggr(out=mv[:], in_=stats[:])
                    rstd = lnp.tile([128, 1], F32, tag="ln1_rstd")
                    rsqrt_cols(rstd[:, 0:1], mv[:, 1:2], 1)
                    hb = lnp.tile([128, D], BF16, tag="hbuf")
                    nc.vector.tensor_scalar(
                        out=hb[:], in0=x_t[:], scalar1=mv[:, 0:1],
                        scalar2=rstd[:, 0:1], op0=ALU.subtract, op1=ALU.mult)
                    ptf = accp.tile([128, D], BF16, tag="ptf")
                    for d in range(ND):
                        nc.tensor.transpose(ptf[:, d * 128:(d + 1) * 128],
                                            hb[:, d * 128:(d + 1) * 128],
                                            ident[:])
                    nc.vector.tensor_copy(
                        out=st['hT'][:, :, twl * 128:(twl + 1) * 128],
                        in_=ptf[:].rearrange("p (d t) -> p d t", t=128))
                return run

            def u_qk(j):
                def run():
                    lo = max(sp0, q_start)
                    hi = min(sp0 + w, q_start + q_len)
                    pq = accp.tile([128, 512], F32, tag="acc")
                    for dd in range(ND // 2):
                        nc.tensor.matmul(
                            pq[:, 0:w],
                            wqk[:, 2 * dd:2 * dd + 2, j * 256:j * 256 + 128],
                            st['hT'][:, 2 * dd:2 * dd + 2, 0:w],
                            start=(dd == 0), stop=(dd == ND // 2 - 1),
                            perf_mode=DR)
                    if lo < hi:
                        nc.vector.tensor_scalar(
                            out=qT2[:, j, lo - q_start:hi - q_start],
                            in0=pq[:, lo - sp0:hi - sp0],
                            scalar1=1.0 / WS, scalar2=bqp[:, j:j + 1],
                            op0=ALU.mult, op1=ALU.add)
                    pk = accp.tile([128, 512], F32, tag="acc")
                    for dd in range(ND // 2):
                        nc.tensor.matmul(
                            pk[:, 0:w],
                            wqk[:, 2 * dd:2 * dd + 2,
                                j * 256 + 128:j * 256 + 256],
                            st['hT'][:, 2 * dd:2 * dd + 2, 0:w],
                            start=(dd == 0), stop=(dd == ND // 2 - 1),
                            perf_mode=DR)
                    nc.scalar.activation(
                        out=kT2[:, j, sp0:sp0 + w], in_=pk[:, 0:w],
                        func=AF.Identity, bias=bkp[:, j:j + 1], scale=1.0 / WS)
                return run

            def u_v(twl, hf):
                def run():
                    half = NH * HD // 2
                    tw = sp0 // 128 + twl
                    v6 = vaug[:, tw].rearrange("p (g t) c -> p g t c", t=2)
                    pv = accp.tile([128, 512], F32, tag="acc")
                    for dd in range(ND // 2):
                        nc.tensor.matmul(
                            pv[:, 0:half],
                            st['hT'][:, 2 * dd:2 * dd + 2,
                                     twl * 128:(twl + 1) * 128],
                            wv[:, 2 * dd:2 * dd + 2, hf * half:(hf + 1) * half],
                            start=(dd == 0), stop=(dd == ND // 2 - 1),
                            perf_mode=DR)
                    pv4 = pv[:, 0:half].rearrange("p (g t c) -> p g t c",
                                                  t=2, c=HD)
                    bv4 = bv_bc[:, hf * half:(hf + 1) * half].rearrange(
                        "p (g t c) -> p g t c", t=2, c=HD)
                    gsl = slice(hf * 3, (hf + 1) * 3)
                    nc.vector.scalar_tensor_tensor(
                        out=v6[:, gsl, 0, 0:HD], in0=pv4[:, :, 0, :],
                        scalar=1.0 / WS, in1=bv4[:, :, 0, :],
                        op0=ALU.mult, op1=ALU.add)
                    nc.vector.scalar_tensor_tensor(
                        out=v6[:, gsl, 1, HD:128], in0=pv4[:, :, 1, :],
                        scalar=1.0 / WS, in1=bv4[:, :, 1, :],
                        op0=ALU.mult, op1=ALU.add)
                    if hf == 1:
                        nc.gpsimd.memset(v6[:, :, 0, HD:128], 1.0)
                        nc.gpsimd.memset(v6[:, :, 1, 0:HD], 1.0)
                return run

            qks = [u_ln(t) for t in range(ntile)] + [u_qk(j) for j in range(NP)]
            for ui, u in enumerate(qks):
                qk_q.append((u, sp0 + w if ui == len(qks) - 1 else sp0, sp0))
            vs = [u_v(t, hf) for t in range(ntile) for hf in range(2)]
            for ui, u in enumerate(vs):
                v_q.append((u, sp0 + w if ui == len(vs) - 1 else sp0))

        for (sp0, w) in [(s, min(512, kv_len - s))
                         for s in range(0, kv_len, 512)]:
            make_ab_units(sp0, w)

        qk_cov = 0
        v_cov = 0
        switched = [False]
        W = {}

        def switch_weights():
            """qkv weights done -> free them, fetch FFN weights."""
            htp_cm.__exit__(None, None, None)
            wearly_cm.__exit__(None, None, None)
            cm = tc.tile_pool(name="wffn", bufs=1, side="right")
            wffn = cm.__enter__()
            W['cm'] = cm
            W['pool'] = wffn
            W['w2'] = wffn.tile([128, NI, D], BF16, tag="w2_sb", name="w2_sb")
            nc.scalar.dma_start(out=W['w2'][:], in_=w2b[:, :])

        def maybe_switch():
            if not qk_q and not v_q and not switched[0]:
                switched[0] = True
                switch_weights()

        def pop_qk():
            nonlocal qk_cov
            u, cov, usp0 = qk_q.popleft()
            # hT ring invariant (bufs=2): span s's tile allocation waits on
            # span s-2's consumers (its v units) -- those must be emitted
            # first or the in-order PE queue deadlocks.
            while v_q and v_cov < usp0 - 512:
                pop_v()
            u()
            qk_cov = max(qk_cov, cov)
            maybe_switch()

        def pop_v():
            nonlocal v_cov
            u, cov = v_q.popleft()
            u()
            v_cov = max(v_cov, cov)
            maybe_switch()

        ffn_q = collections.deque()

        def pump(n=1):
            for _ in range(n):
                # alternate qk/v production, keeping v within a span of qk
                if qk_q and (not v_q or qk_cov - v_cov <= 512):
                    pop_qk()
                elif v_q:
                    pop_v()
                elif ffn_q:
                    ffn_q.popleft()()
                else:
                    break

        def flush_ffn():
            while qk_q:
                pop_qk()
            while v_q:
                pop_v()
            while ffn_q:
                ffn_q.popleft()()

        # ---------------- FFN units (LN2 + ff1 + ff2) per chunk ------------
        def make_ffn_units(cc0, cw):
            ctws = cw // 128
            tb = cc0 // 128
            st = {}

            def u_stats():
                for twl in range(ctws):
                    tw = tb + twl
                    stats = lnp.tile([128, D // BN_SUB, 6], F32,
                                     tag="ln2_stats")
                    xs = oaT[:, tw, :].rearrange("p (s c) -> p s c", c=BN_SUB)
                    for sgi in range(D // BN_SUB):
                        nc.vector.bn_stats(out=stats[:, sgi, :],
                                           in_=xs[:, sgi, :])
                    nc.vector.bn_aggr(out=mv2[:, tw, :], in_=stats[:])
                var = mv2[:, tb:tb + ctws, 1:2].rearrange("p a b -> p (a b)")
                rsqrt_cols(rsds[:, tb:tb + ctws], var, ctws)

            def u_tp(twl):
                def run():
                    tw = tb + twl
                    if 'h2T' not in st:
                        st['h2T'] = h2_pool.tile([128, ND, 512], BF16,
                                                 tag="h2T", name="h2T")
                    hb = lnp.tile([128, D], BF16, tag="hbuf")
                    nc.vector.tensor_scalar(
                        out=hb[:], in0=oaT[:, tw, :], scalar1=mv2[:, tw, 0:1],
                        scalar2=rsds[:, tw:tw + 1],
                        op0=ALU.subtract, op1=ALU.mult)
                    ptf = accp.tile([128, D], BF16, tag="ptf")
                    for d in range(ND):
                        nc.tensor.transpose(
                            ptf[:, d * 128:(d + 1) * 128],
                            hb[:, d * 128:(d + 1) * 128], ident[:])
                    nc.vector.tensor_copy(
                        out=st['h2T'][:, :, twl * 128:(twl + 1) * 128],
                        in_=ptf[:].rearrange("p (d t) -> p d t", t=128))
                return run

            def u_ff1(i_):
                def run():
                    if 'ff1' not in st:
                        st['ff1'] = W['pool'].tile([128, NI, 512], BF16,
                                                   tag="ff1", name="ff1")
                    pf = accp.tile([128, 512], F32, tag="acc")
                    for d in range(ND):
                        nc.tensor.matmul(
                            pf[:, 0:cw],
                            w1_sb[:, d, i_ * 128:(i_ + 1) * 128],
                            st['h2T'][:, d, 0:cw],
                            start=(d == 0), stop=(d == ND - 1))
                    if s_idx == 0:
                        nc.scalar.activation(
                            out=st['ff1'][:, i_, 0:cw], in_=pf[:, 0:cw],
                            func=AF.Identity, bias=b1_sb[:, i_:i_ + 1],
                            scale=1.0)
                    else:
                        nc.vector.tensor_scalar_add(
                            out=st['ff1'][:, i_, 0:cw], in0=pf[:, 0:cw],
                            scalar1=b1_sb[:, i_:i_ + 1])
                return run

            def u_ff2(twl, hf):
                def run():
                    tw = tb + twl
                    pg = accp.tile([128, 512], F32, tag="acc")
                    for i_ in range(NI):
                        nc.tensor.matmul(
                            pg[:, 0:Dh],
                            st['ff1'][:, i_, twl * 128:(twl + 1) * 128],
                            W['w2'][:, i_, hf * Dh:(hf + 1) * Dh],
                            start=(i_ == 0), stop=(i_ == NI - 1))
                    sl = slice(hf * Dh, (hf + 1) * Dh)
                    gb = gbp.tile([128, Dh], BF16, tag="gb")
                    nc.vector.tensor_tensor(out=gb[:], in0=pg[:, 0:Dh],
                                            in1=b2_bc[:, sl], op=ALU.add)
                    nc.sync.dma_start(
                        out=g2d[tw * 128:(tw + 1) * 128, sl], in_=gb[:])
                return run

            ffn_q.append(u_stats)
            for twl in range(ctws):
                ffn_q.append(u_tp(twl))
            for i_ in range(NI):
                ffn_q.append(u_ff1(i_))
            for twl in range(ctws):
                for hf in range(2):
                    ffn_q.append(u_ff2(twl, hf))

        # ---------------- attention span loop ----------------
        for (q0, w, nfull) in attn_spans(q_start, q_len):
            ndiag = w // 128
            ktot = nfull + ndiag
            for j in range(NP):
                h0, h1 = 2 * j, 2 * j + 1
                po0 = ops_.tile([128, 512], F32, tag="po")
                po1 = ops_.tile([128, 512], F32, tag="po")
                po = [po0, po1]

                # ps slot layout along dim1: slot = 2*hh + kki so the PV
                # DoubleRow rhs et[:, 2*hh:2*hh+2, :] is a contiguous pair.
                # Scores of the head pair go to PE row halves 0:64 / 64:128
                # (disjoint row groups -> concurrent matmuls).
                def emit_pv(group):
                    et, tiles = group
                    both_full = (len(tiles) == 2
                                 and all(k < nfull for (k, _, _, _) in tiles))
                    for hh, h in ((0, h0), (1, h1)):
                        for (k, kki, kc0, wj) in tiles:
                            if k >= nfull:
                                nc.vector.tensor_tensor(
                                    out=et[:, 2 * hh + kki, 0:128],
                                    in0=et[:, 2 * hh + kki, 0:128],
                                    in1=tri[:], op=ALU.mult)
                        if both_full:
                            k = tiles[0][0]
                            nc.tensor.matmul(
                                po[hh][:, 0:w],
                                vaug[:, k:k + 2, h, :],
                                et[:, 2 * hh:2 * hh + 2, 0:w],
                                start=(k == 0), stop=(k + 1 == ktot - 1),
                                perf_mode=DR)
                        else:
                            for (k, kki, kc0, wj) in tiles:
                                nc.tensor.matmul(
                                    po[hh][:, kc0:w], vaug[:, k, h, :],
                                    et[:, 2 * hh + kki, 0:wj],
                                    start=(k == 0), stop=(k == ktot - 1))

                prev = None
                kt = 0
                while kt < ktot:
                    ng = min(2, ktot - kt)
                    # make sure the q/k units covering these keys (and this
                    # span's q) are emitted
                    need = max((kt + ng) * 128, q0 + w)
                    while qk_q and qk_cov < need:
                        pop_qk()
                    ps = sps.tile([128, 4, 512], F32, tag="ps")
                    tiles = []
                    for kki in range(ng):
                        k = kt + kki
                        kc0 = max(0, 128 * (k - nfull))
                        wj = w - kc0
                        tiles.append((k, kki, kc0, wj))
                        for hh, hb2 in ((0, 0), (1, 64)):
                            nc.tensor.matmul(
                                ps[:, 2 * hh + kki, 0:wj],
                                kT2[hb2:hb2 + 64, j, k * 128:(k + 1) * 128],
                                qT2[hb2:hb2 + 64, j,
                                    q0 + kc0 - q_start:q0 + w - q_start],
                                start=True, stop=True)
                    wmax = tiles[0][3]
                    et = epool.tile([128, 4, 512], F8, tag="et")
                    nc.scalar.activation(out=et[:, 0:4, 0:wmax],
                                         in_=ps[:, 0:4, 0:wmax],
                                         func=AF.Exp, bias=0.0, scale=1.0 / HD)
                    pump(3)
                    if prev is not None:
                        vneed = (prev[1][-1][0] + 1) * 128
                        while v_q and v_cov < vneed:
                            pop_v()
                        emit_pv(prev)
                    prev = (et, tiles)
                    kt += ng
                vneed = (prev[1][-1][0] + 1) * 128
                while v_q and v_cov < vneed:
                    pop_v()
                emit_pv(prev)
                # softmax normalize: h0 den at po0[64:128], h1 den at po1[0:64]
                dcp = rpool.tile([128, 512], F32, tag="dcp")
                nc.vector.tensor_copy(out=dcp[0:64, 0:w],
                                      in_=po0[64:128, 0:w])
                nc.vector.tensor_copy(out=dcp[64:128, 0:w],
                                      in_=po1[0:64, 0:w])
                rt = rpool.tile([128, 512], F32, tag="rt")
                nc.vector.reciprocal_approx_fast(out=rt[:, 0:w],
                                                 in_=dcp[:, 0:w])
                nc.vector.scalar_tensor_tensor(
                    out=attnT[0:64, j, q0 - q_start:q0 - q_start + w],
                    in0=po0[0:64, 0:w], scalar=AS, in1=rt[0:64, 0:w],
                    op0=ALU.mult, op1=ALU.mult)
                nc.vector.scalar_tensor_tensor(
                    out=attnT[64:128, j, q0 - q_start:q0 - q_start + w],
                    in0=po1[64:128, 0:w], scalar=AS, in1=rt[64:128, 0:w],
                    op0=ALU.mult, op1=ALU.mult)

            # D1 for this span: w_o + residual (+b_o) -> oaT (bf16)
            for twl in range(w // 128):
                tw = (q0 - q_start) // 128 + twl
                xo = xpool.tile([128, D], F32, tag="xo")
                nc.sync.dma_start(
                    out=xo[:],
                    in_=x[q_start + tw * 128:q_start + (tw + 1) * 128, :])
                nc.vector.tensor_tensor(out=xo[:], in0=xo[:], in1=bo_bc[:],
                                        op=ALU.add)
                pump(2)
                for hf in range(2):
                    pwt = ops_.tile([128, 512], F32, tag="po")
                    for aa in range(ND // 2):
                        nc.tensor.matmul(
                            pwt[:, 0:Dh],
                            attnT[:, 2 * aa:2 * aa + 2,
                                  tw * 128:(tw + 1) * 128],
                            wo_sb[:, 2 * aa:2 * aa + 2,
                                  hf * Dh:(hf + 1) * Dh],
                            start=(aa == 0), stop=(aa == ND // 2 - 1),
                            perf_mode=DR)
                    nc.vector.scalar_tensor_tensor(
                        out=oaT[:, tw, hf * Dh:(hf + 1) * Dh],
                        in0=pwt[:, 0:Dh], scalar=1.0 / (WS * AS),
                        in1=xo[:, hf * Dh:(hf + 1) * Dh],
                        op0=ALU.mult, op1=ALU.add)
            # queue this span's FFN; it gets pumped during the next span
            make_ffn_units(q0 - q_start, w)

        flush_ffn()

        # ---- tail: gelu (one table load) + residual + store
        # (tiles reuse existing same-shape tags to save SBUF arena)
        for tw in range(NT_Q):
            gin = lnp.tile([128, D], BF16, tag="hbuf")
            nc.sync.dma_start(out=gin[:],
                              in_=g2d[tw * 128:(tw + 1) * 128, :])
            gf = xpool.tile([128, D], F32, tag="x_t")
            nc.scalar.activation(out=gf[:], in_=gin[:], func=AF.Gelu,
                                 bias=0.0, scale=1.0)
            nc.vector.tensor_tensor(out=gf[:], in0=gf[:], in1=oaT[:, tw, :],
                                    op=ALU.add)
            nc.sync.dma_start(out=out[tw * 128:(tw + 1) * 128, :], in_=gf[:])

        W['cm'].__exit__(None, None, None)


_NC_CACHE = {}


def build_kernel():
    key = (B, L, D, H, I, T0)
    if key in _NC_CACHE:
        return _NC_CACHE[key]
    nc = bacc.Bacc("TRN2", target_bir_lowering=False, debug=False,
                   num_devices=N_CORES)
    NP = NH // 2
    x = nc.dram_tensor("x", [L, D], F32, kind="ExternalInput").ap()
    wqkb = nc.dram_tensor("wqkb", [128, ND * NP * 256], F8,
                          kind="ExternalInput").ap()
    wvb = nc.dram_tensor("wvb", [128, ND * NH * HD], F8,
                         kind="ExternalInput").ap()
    wo8 = nc.dram_tensor("wo8", [128, ND * D], F8, kind="ExternalInput").ap()
    w1b = nc.dram_tensor("w1b", [128, ND * I], BF16, kind="ExternalInput").ap()
    w2b = nc.dram_tensor("w2b", [128, NI * D], BF16, kind="ExternalInput").ap()
    bq_t = nc.dram_tensor("bq_t", [128, NP], F32, kind="ExternalInput").ap()
    bk_t = nc.dram_tensor("bk_t", [128, NP], F32, kind="ExternalInput").ap()
    bv = nc.dram_tensor("bv", [NH * HD], F32, kind="ExternalInput").ap()
    bo = nc.dram_tensor("b_o", [D], F32, kind="ExternalInput").ap()
    b1t = nc.dram_tensor("b1t", [128, NI], F32, kind="ExternalInput").ap()
    b2 = nc.dram_tensor("b2", [D], F32, kind="ExternalInput").ap()
    out = nc.dram_tensor("out", [T0, D], F32, kind="ExternalOutput").ap()
    io = (x, wqkb, wvb, wo8, w1b, w2b, bq_t, bk_t, bv, bo, b1t, b2, out)

    pid = nc.partition_id()
    with tile.TileContext(nc) as tc:
        with tc.If(pid % 2 == 0):
            with contextlib.ExitStack() as c0:
                build_body(nc, tc, c0, io, 0, T0, T0, 0)
        with tc.If(pid % 2 == 1):
            with contextlib.ExitStack() as c1:
                build_body(nc, tc, c1, io, T0, T1, L, 1)
    nc.compile()
    _NC_CACHE[key] = nc
    return nc


def make_in_maps(inputs):
    """Fold LN gains/biases into the adjacent projection weights (exact in
    fp32 terms), pre-gather + cast weight layouts, and build the per-core
    input maps."""
    F8NP = ml_dtypes.float8_e4m3
    BFNP = ml_dtypes.bfloat16
    NP = NH // 2
    x = np.asarray(inputs["x"], dtype=np.float32)
    am = np.asarray(inputs["attention_mask"])
    assert am.all(), "kernel assumes attention_mask all-True (spec fill=ones)"
    g = {n: np.asarray(inputs[n], np.float64)
         for n in ["w_qkv", "b_qkv", "w_o", "b_o", "w1", "b1", "w2", "b2",
                   "ln1_g", "ln1_b", "ln2_g", "ln2_b"]}
    wqkv = (g["ln1_g"][:, None] * g["w_qkv"]).astype(np.float32)  # [D, 3D]
    bqkv = (g["ln1_b"] @ g["w_qkv"] + g["b_qkv"]).astype(np.float32)
    w1 = (g["ln2_g"][:, None] * g["w1"]).astype(np.float32)
    b1 = (g["ln2_b"] @ g["w1"] + g["b1"]).astype(np.float32)
    w_o = g["w_o"].astype(np.float32)

    # per-head qkv chunks: head n occupies cols [3*HD*n, 3*HD*(n+1)) as
    # [q(64) | k(64) | v(64)].
    wq4 = wqkv.reshape(ND, 128, NP, 2, 3 * HD)        # [d, p, j, t, c]
    # pair-packed: per pair j the 256 cols are [q_h0 | q_h1 | k_h0 | k_h1]
    wqkb = np.concatenate(
        [wq4[:, :, :, 0, 0:HD], wq4[:, :, :, 1, 0:HD],
         wq4[:, :, :, 0, HD:2 * HD], wq4[:, :, :, 1, HD:2 * HD]],
        axis=3)                                        # [d, p, j, 256]
    wqkb = np.ascontiguousarray(
        np.clip(wqkb.transpose(1, 0, 2, 3) * WS, -240, 240)
    ).astype(F8NP).reshape(128, -1)
    wvb = np.ascontiguousarray(
        np.clip(wq4[:, :, :, :, 2 * HD:].reshape(ND, 128, NH, HD).transpose(
            1, 0, 2, 3) * WS, -240, 240)
    ).astype(F8NP).reshape(128, -1)
    bq4 = bqkv.reshape(NP, 2, 3 * HD)
    bq_t = np.ascontiguousarray(
        np.concatenate([bq4[:, 0, 0:HD], bq4[:, 1, 0:HD]], axis=1).T)
    bk_t = np.ascontiguousarray(
        np.concatenate([bq4[:, 0, HD:2 * HD], bq4[:, 1, HD:2 * HD]], axis=1).T)
    bv = np.ascontiguousarray(bq4[:, :, 2 * HD:].reshape(-1))
    wo8 = np.ascontiguousarray(
        np.clip(w_o.reshape(ND, 128, D).transpose(1, 0, 2) * WS, -240, 240)
    ).astype(F8NP).reshape(128, ND * D)
    w1b = np.ascontiguousarray(
        w1.reshape(ND, 128, I).transpose(1, 0, 2)).astype(BFNP).reshape(
        128, ND * I)
    w2b = np.ascontiguousarray(
        g["w2"].astype(np.float32).reshape(NI, 128, D).transpose(1, 0, 2)
    ).astype(BFNP).reshape(128, NI * D)
    b1t = np.ascontiguousarray(b1.reshape(NI, 128).T)  # [128, NI]

    common = {
        "wqkb": wqkb, "wvb": wvb, "bq_t": bq_t, "bk_t": bk_t, "bv": bv,
        "wo8": wo8, "b_o": g["b_o"].astype(np.float32),
        "w1b": w1b, "b1t": b1t, "w2b": w2b, "b2": g["b2"].astype(np.float32),
    }
    in_maps = []
    for pid in range(N_CORES):
        b = pid // 2
        m = dict(common)
        m["x"] = np.ascontiguousarray(x[b])
        in_maps.append(m)
    return in_maps


def kernel(**inputs):
    nc = build_kernel()
    in_maps = make_in_maps(inputs)
    res = run_bass_kernel_spmd(nc, in_maps, core_ids=list(range(N_CORES)))
    out = np.empty((B, L, D), np.float32)
    for b in range(B):
        out[b, :T0] = res.results[2 * b]["out"][:T0]
        out[b, T0:] = res.results[2 * b + 1]["out"][:T1]
    return out


if __name__ == "__main__":
    rng = np.random.default_rng(0)
    ins = {
        "x": rng.standard_normal((B, L, D)).astype(np.float32),
        "attention_mask": np.ones((B, L), bool),
        "ln1_g": np.ones(D, np.float32), "ln1_b": np.zeros(D, np.float32),
        "w_qkv": (rng.standard_normal((D, 3 * D)) * 0.02).astype(np.float32),
        "b_qkv": np.zeros(3 * D, np.float32),
        "w_o": (rng.standard_normal((D, D)) * 0.02).astype(np.float32),
        "b_o": np.zeros(D, np.float32),
        "ln2_g": np.ones(D, np.float32), "ln2_b": np.zeros(D, np.float32),
        "w1": (rng.standard_normal((D, I)) * 0.02).astype(np.float32),
        "b1": np.zeros(I, np.float32),
        "w2": (rng.standard_normal((I, D)) * 0.02).astype(np.float32),
        "b2": np.zeros(D, np.float32),
    }
    o = kernel(**ins)
    print("kernel out:", o.shape, o.dtype, np.abs(o).max())
